# revision 1
# baseline (speedup 1.0000x reference)
"""Trainium2 Bass kernel for a 2-layer GCN encoder + global mean pool.

Reference computation (PyG GCNConv semantics, eval mode):
    h1 = relu(Ahat @ (x @ W1) + b1)
    h2 = Ahat @ (h1 @ W2) + b2          (aggregation reordered: Ahat@(h1) then @W2)
    out = segment_mean(h2, batch)        -> [NUM_GRAPHS, OUT_DIM]
with Ahat = D^-1/2 (A + I) D^-1/2, deg = in-degree + 1.

Strategy (8 NeuronCores, SPMD):
  - Nodes (rows) sharded contiguously across cores; incident edges assigned to
    the core owning their dst row.  Weights replicated.
  - Dense matmul X@W1 done shard-wise (DMA-transposed x tiles feed the PE).
  - The normalized message table y = dis * h is AllGathered so every core can
    gather arbitrary src rows with indirect DMA.
  - Edge aggregation: host sorts edges by dst block (128 dst rows per block),
    pads each block to a uniform number of 128-edge chunks.  Per chunk a 0/1
    one-hot matrix (built on DVE from slot ids vs an iota tile) maps edges to
    dst slots; PE matmuls accumulate messages into PSUM per block.  Self-loops
    are appended as ordinary edges.
  - Layer 2 aggregates in the transposed orientation so the gathered tile is
    the stationary operand, then applies W2 on-chip.
  - Mean pool: segmented one-hot matmul over node chunks (batch is sorted);
    per-core partial sums are returned and combined on the host.
"""

import math
import os

import ml_dtypes
import numpy as np

P = 128
N_NODES = 100000
N_EDGES = 1600000
NUM_GRAPHS = 1000
IN_DIM, HID_DIM, OUT_DIM = 256, 128, 64
N_CORES = 8

BF16 = ml_dtypes.bfloat16
PAD_SLOT = 255.0  # one-hot build never matches iota 0..127


class Plan:
    """Host-side preprocessing result: all per-core arrays + layout constants."""


def make_plan(x, W1, b1, W2, b2, edge_index, batch,
              n_nodes=N_NODES, num_graphs=NUM_GRAPHS, n_cores=N_CORES):
    pl = Plan()
    n_pc = n_nodes // n_cores
    assert n_pc * n_cores == n_nodes
    n_blk = math.ceil(n_pc / P)
    n_pad = n_blk * P
    pl.n_nodes, pl.num_graphs, pl.n_cores = n_nodes, num_graphs, n_cores
    pl.n_pc, pl.n_blk, pl.n_pad = n_pc, n_blk, n_pad
    pl.d_in, pl.d_hid, pl.d_out = x.shape[1], W1.shape[1], W2.shape[1]

    src = np.asarray(edge_index[0], dtype=np.int64)
    dst = np.asarray(edge_index[1], dtype=np.int64)
    batch = np.asarray(batch, dtype=np.int64)

    deg = np.bincount(dst, minlength=n_nodes).astype(np.float64) + 1.0
    dis = (1.0 / np.sqrt(deg)).astype(np.float32)

    # self-loops are handled densely per block (identity matmul on the
    # core's own shard rows), not via the gather grid
    srcs = src
    dsts = dst

    core = dsts // n_pc
    loc = dsts - core * n_pc
    blk = loc // P
    slot = loc % P

    key = core * n_blk + blk
    order = np.argsort(key, kind="stable")
    counts = np.bincount(key, minlength=n_cores * n_blk)
    # per-block chunk count: max over cores (uniform across cores for SPMD)
    C_g = np.ceil(counts.reshape(n_cores, n_blk).max(axis=0) / P).astype(np.int64)
    C_g = np.maximum(C_g, 1)
    col_base = np.concatenate([[0], np.cumsum(C_g)])
    C_tot = int(col_base[-1])
    pl.C_g, pl.col_base, pl.C_tot = C_g, col_base, C_tot

    sorted_key = key[order]
    block_start = np.concatenate([[0], np.cumsum(counts)])[:-1]
    rank = np.arange(len(order)) - block_start[sorted_key]
    pp = rank % P
    cc = rank // P
    blk_o = blk[order]
    core_o = core[order]
    col = col_base[blk_o] + cc

    half = n_pc // 2
    upper_base = n_cores * half
    so = srcs[order]
    core_s = so // n_pc
    loc_s = so - core_s * n_pc
    row_remap = np.where(loc_s < half,
                         core_s * half + loc_s,
                         upper_base + core_s * (n_pc - half) + (loc_s - half))
    pl.ag_half = half
    idx_all = np.zeros((n_cores, P, C_tot), dtype=np.int32)
    slots_all = np.full((n_cores, P, C_tot), PAD_SLOT, dtype=BF16)
    idx_all[core_o, pp, col] = row_remap.astype(np.int32)
    slots_all[core_o, pp, col] = slot[order].astype(BF16)
    pl.idx_all, pl.slots_all = idx_all, slots_all

    # per-node scalars laid out [core][P, n_blk] (partition p, block g)
    def node_layout(vals, pad=0.0):
        out = np.full((n_cores, P, n_blk), pad, dtype=np.float32)
        v = vals.reshape(n_cores, n_pc)
        for k in range(n_cores):
            full = np.full(n_pad, pad, dtype=np.float32)
            full[:n_pc] = v[k]
            out[k] = full.reshape(n_blk, P).T
        return out

    pl.dis_t = node_layout(dis)
    cnt = np.bincount(batch, minlength=num_graphs).astype(np.float64)
    recip_g = (1.0 / np.maximum(cnt, 1.0)).astype(np.float32)
    pl.recip_t = node_layout(recip_g[batch])

    # pooling: groups of G_CH node-chunks share a PSUM tile; slot = graph - base
    G_CH = 13
    while True:
        n_grp = math.ceil(n_blk / G_CH)
        ok = True
        pool_slots = np.full((n_cores, P, n_blk), PAD_SLOT, dtype=np.float32)
        pool_base = np.zeros((n_cores, n_grp), dtype=np.int64)
        for k in range(n_cores):
            b = batch[k * n_pc:(k + 1) * n_pc]
            for g in range(n_grp):
                lo = g * G_CH * P
                if lo >= n_pc:
                    pool_base[k, g] = 0
                    continue
                hi = min((g + 1) * G_CH * P, n_pc)
                base = b[lo]
                pool_base[k, g] = base
                rel = b[lo:hi] - base
                if rel.max() >= P:
                    ok = False
                    break
                sl = np.full(min((g + 1) * G_CH * P, n_blk * P) - lo, PAD_SLOT,
                             dtype=np.float32)
                sl[:hi - lo] = rel
                dstv = pool_slots[k].T.reshape(-1)
                dstv[lo:lo + len(sl)] = sl
                pool_slots[k] = dstv.reshape(n_blk, P).T
            if not ok:
                break
        if ok:
            break
        G_CH //= 2
        assert G_CH >= 1
    pl.G_CH, pl.n_grp = G_CH, n_grp
    pl.pool_slots, pl.pool_base = pool_slots, pool_base

    # x shards (padded rows), bf16
    x = np.asarray(x, dtype=np.float32)
    x_sh = np.zeros((n_cores, n_pad, pl.d_in), dtype=BF16)
    x_sh[:, :n_pc] = x.reshape(n_cores, n_pc, pl.d_in).astype(BF16)
    pl.x_sh = x_sh

    # w1 packed [P, 2*d_hid]: w1t[p, k*d_hid + n] = W1[k*P + p, n]
    W1 = np.asarray(W1, dtype=np.float32)
    W2 = np.asarray(W2, dtype=np.float32)
    kk = pl.d_in // P
    pl.w1t = np.concatenate([W1[k * P:(k + 1) * P] for k in range(kk)],
                            axis=1).astype(BF16)  # [P, kk*d_hid]
    pl.n_k1 = kk
    pl.w2_sb = W2.astype(BF16)  # [d_hid, d_out], d_hid == P

    pl.b1b = np.broadcast_to(np.asarray(b1, np.float32), (P, pl.d_hid)).copy()
    pl.b2b = np.broadcast_to(np.asarray(b2, np.float32), (P, pl.d_out)).copy()
    iot = np.broadcast_to(np.arange(P, dtype=np.float32), (P, P))
    pl.iotab = iot.astype(BF16).copy()
    pl.iotaf = iot.astype(np.float32).copy()
    pl.ident = np.eye(P, dtype=BF16)
    return pl


def build_program(pl, body_repeat=1):
    import concourse.bass as bass
    import concourse.mybir as mybir
    import concourse.tile as tile
    from concourse import bacc

    f32 = mybir.dt.float32
    bf16 = mybir.dt.bfloat16
    i32 = mybir.dt.int32
    AF = mybir.ActivationFunctionType
    OP = mybir.AluOpType

    n_pc, n_blk, n_pad = pl.n_pc, pl.n_blk, pl.n_pad
    C_g, col_base, C_tot = pl.C_g, pl.col_base, pl.C_tot
    d_in, d_hid, d_out = pl.d_in, pl.d_hid, pl.d_out
    n_cores = pl.n_cores

    nc = bacc.Bacc("TRN2", target_bir_lowering=False, debug=False,
                   num_devices=n_cores)

    # --- I/O ---
    x_sh = nc.dram_tensor("x_sh", [n_pad, d_in], bf16, kind="ExternalInput")
    w1t_d = nc.dram_tensor("w1t", [P, pl.n_k1 * d_hid], bf16, kind="ExternalInput")
    w2_d = nc.dram_tensor("w2", [d_hid, d_out], bf16, kind="ExternalInput")
    b1b_d = nc.dram_tensor("b1b", [P, d_hid], f32, kind="ExternalInput")
    b2b_d = nc.dram_tensor("b2b", [P, d_out], f32, kind="ExternalInput")
    iotab_d = nc.dram_tensor("iotab", [P, P], bf16, kind="ExternalInput")
    iotaf_d = nc.dram_tensor("iotaf", [P, P], f32, kind="ExternalInput")
    dis_d = nc.dram_tensor("dis_t", [P, n_blk], f32, kind="ExternalInput")
    recip_d = nc.dram_tensor("recip_t", [P, n_blk], f32, kind="ExternalInput")
    idx_d = nc.dram_tensor("idx_all", [P, C_tot], i32, kind="ExternalInput")
    slots_d = nc.dram_tensor("slots_all", [P, C_tot], bf16, kind="ExternalInput")
    pslots_d = nc.dram_tensor("pool_slots", [P, n_blk], f32, kind="ExternalInput")
    ident_d = nc.dram_tensor("ident", [P, P], bf16, kind="ExternalInput")

    pool_part = nc.dram_tensor("pool_part", [pl.n_grp * P, d_out], f32,
                               kind="ExternalOutput")

    # --- internal DRAM ---
    y1_sh = nc.dram_tensor("y1_sh", [n_pad, d_hid], bf16)
    z1_sh = nc.dram_tensor("z1_sh", [n_pad, d_hid], bf16)
    y1_full = nc.dram_tensor("y1_full", [pl.n_nodes, d_hid], bf16,
                             addr_space="Shared")
    y2_full = nc.dram_tensor("y2_full", [pl.n_nodes, d_hid], bf16,
                             addr_space="Shared")
    z2_dram = nc.dram_tensor("z2_d", [n_pad, d_out], f32)

    groups = [list(range(n_cores))]

    with tile.TileContext(nc) as tc:
        with (
            tc.tile_pool(name="const", bufs=1) as cpool,
            tc.tile_pool(name="sb", bufs=5) as sb,
            tc.tile_pool(name="sb2", bufs=3) as sb2,
            tc.tile_pool(name="ps_agg", bufs=2, space="PSUM") as ps_agg,
            tc.tile_pool(name="ps_fe", bufs=2, space="PSUM") as ps_fe,
            tc.tile_pool(name="ps_o", bufs=2, space="PSUM") as ps_o,
            tc.tile_pool(name="ps_p", bufs=2, space="PSUM") as ps_p,
        ):
            # persistent constants
            w1_sb = cpool.tile([P, pl.n_k1 * d_hid], bf16)
            nc.sync.dma_start(out=w1_sb[:], in_=w1t_d[:, :])
            w2_sb = cpool.tile([d_hid, d_out], bf16)
            nc.sync.dma_start(out=w2_sb[:], in_=w2_d[:, :])
            b1_sb = cpool.tile([P, d_hid], f32)
            nc.sync.dma_start(out=b1_sb[:], in_=b1b_d[:, :])
            b2_sb = cpool.tile([P, d_out], f32)
            nc.sync.dma_start(out=b2_sb[:], in_=b2b_d[:, :])
            iob_sb = cpool.tile([P, P], bf16)
            nc.sync.dma_start(out=iob_sb[:], in_=iotab_d[:, :])
            iof_sb = cpool.tile([P, P], f32)
            nc.sync.dma_start(out=iof_sb[:], in_=iotaf_d[:, :])
            dis_sb = cpool.tile([P, n_blk], f32)
            nc.sync.dma_start(out=dis_sb[:], in_=dis_d[:, :])
            recip_sb = cpool.tile([P, n_blk], f32)
            nc.sync.dma_start(out=recip_sb[:], in_=recip_d[:, :])
            idx_sb = cpool.tile([P, C_tot], i32)
            nc.sync.dma_start(out=idx_sb[:], in_=idx_d[:, :])
            slots_sb = cpool.tile([P, C_tot], bf16)
            nc.sync.dma_start(out=slots_sb[:], in_=slots_d[:, :])
            pslots_sb = cpool.tile([P, n_blk], f32)
            nc.sync.dma_start(out=pslots_sb[:], in_=pslots_d[:, :])
            ident_sb = cpool.tile([P, P], bf16)
            nc.sync.dma_start(out=ident_sb[:], in_=ident_d[:, :])

            for _rep in range(body_repeat):
                # ---------- front-end: y1 = dis * (x @ W1), bf16 ----------
                for g in range(n_blk):
                    psum_h = ps_fe.tile([P, d_hid], f32, tag="feps")
                    for k in range(pl.n_k1):
                        xT = sb.tile([P, P], bf16, tag="xT")
                        nc.sync.dma_start(
                            out=xT[:],
                            in_=x_sh[g * P:(g + 1) * P, k * P:(k + 1) * P],
                            transpose=True)
                        nc.tensor.matmul(psum_h[:], lhsT=xT[:],
                                         rhs=w1_sb[:, k * d_hid:(k + 1) * d_hid],
                                         start=(k == 0), stop=(k == pl.n_k1 - 1))
                    y1t = sb.tile([P, d_hid], bf16, tag="y1t")
                    nc.scalar.activation(y1t[:], psum_h[:], AF.Copy,
                                         scale=dis_sb[:, g:g + 1])
                    nc.sync.dma_start(out=y1_sh[g * P:(g + 1) * P, :], in_=y1t[:])

                half = pl.ag_half
                ub = n_cores * half
                nc.gpsimd.collective_compute(
                    "AllGather", OP.bypass, replica_groups=groups,
                    ins=[y1_sh[0:half, :]], outs=[y1_full[0:ub, :]])
                nc.gpsimd.collective_compute(
                    "AllGather", OP.bypass, replica_groups=groups,
                    ins=[y1_sh[half:n_pc, :]], outs=[y1_full[ub:pl.n_nodes, :]])

                # ---------- layer aggregation ----------
                def gather_block(g, y_full):
                    cg = int(C_g[g])
                    base = int(col_base[g])
                    gt = sb.tile([P, cg * d_hid], bf16, tag="gath")
                    # HW indirect DMA semantics: one offset per partition per op
                    # (gathers dst[p, :] = src[idx[p]]), so issue one op per chunk.
                    for c in range(cg):
                        nc.gpsimd.indirect_dma_start(
                            out=gt[:, c * d_hid:(c + 1) * d_hid],
                            out_offset=None,
                            in_=y_full[:, :],
                            in_offset=bass.IndirectOffsetOnAxis(
                                ap=idx_sb[:, base + c:base + c + 1], axis=0))
                    mt = sb.tile([P, cg * P], bf16, tag="onehot")
                    nc.vector.tensor_tensor(
                        out=mt[:].rearrange("p (c q) -> p c q", q=P),
                        in0=slots_sb[:, base:base + cg].to_broadcast([P, cg, P]),
                        in1=iob_sb[:, None, :].to_broadcast([P, cg, P]),
                        op=OP.is_equal)
                    return gt, mt

                # layer 1: psum[slot, feat] += M_c.T @ G_c
                for g in range(n_blk):
                    gt, mt = gather_block(g, y1_full)
                    cg = int(C_g[g])
                    sl1 = sb.tile([P, d_hid], bf16, tag="sloop")
                    nc.sync.dma_start(out=sl1[:],
                                      in_=y1_sh[g * P:(g + 1) * P, :])
                    psum_a = ps_agg.tile([P, d_hid], f32, tag="agg")
                    nc.tensor.matmul(psum_a[:], lhsT=ident_sb[:], rhs=sl1[:],
                                     start=True, stop=False)
                    for c in range(cg):
                        nc.tensor.matmul(psum_a[:],
                                         lhsT=mt[:, c * P:(c + 1) * P],
                                         rhs=gt[:, c * d_hid:(c + 1) * d_hid],
                                         start=False, stop=(c == cg - 1))
                    t1 = sb2.tile([P, d_hid], f32, tag="ep1")
                    nc.scalar.activation(t1[:], psum_a[:], AF.Copy,
                                         scale=dis_sb[:, g:g + 1])
                    t2 = sb2.tile([P, d_hid], f32, tag="ep2")
                    nc.vector.tensor_tensor(t2[:], t1[:], b1_sb[:], op=OP.add)
                    z1t = sb2.tile([P, d_hid], bf16, tag="z1t")
                    nc.vector.tensor_scalar(out=z1t[:], in0=t2[:],
                                            scalar1=0.0, scalar2=dis_sb[:, g:g + 1],
                                            op0=OP.max, op1=OP.mult)
                    nc.sync.dma_start(out=z1_sh[g * P:(g + 1) * P, :], in_=z1t[:])

                nc.gpsimd.collective_compute(
                    "AllGather", OP.bypass, replica_groups=groups,
                    ins=[z1_sh[0:half, :]], outs=[y2_full[0:ub, :]])
                nc.gpsimd.collective_compute(
                    "AllGather", OP.bypass, replica_groups=groups,
                    ins=[z1_sh[half:n_pc, :]], outs=[y2_full[ub:pl.n_nodes, :]])

                # layer 2: psumT[feat, slot] += G_c.T @ M_c ; then @ W2
                for g in range(n_blk):
                    gt, mt = gather_block(g, y2_full)
                    cg = int(C_g[g])
                    sl2 = sb.tile([P, d_hid], bf16, tag="sloop")
                    nc.sync.dma_start(out=sl2[:],
                                      in_=z1_sh[g * P:(g + 1) * P, :])
                    psum_t = ps_agg.tile([P, P], f32, tag="agg")
                    nc.tensor.matmul(psum_t[:], lhsT=sl2[:], rhs=ident_sb[:],
                                     start=True, stop=False)
                    for c in range(cg):
                        nc.tensor.matmul(psum_t[:],
                                         lhsT=gt[:, c * d_hid:(c + 1) * d_hid],
                                         rhs=mt[:, c * P:(c + 1) * P],
                                         start=False, stop=(c == cg - 1))
                    s2t = sb2.tile([P, P], bf16, tag="s2t")
                    nc.scalar.activation(s2t[:], psum_t[:], AF.Copy)
                    psum_o = ps_o.tile([P, d_out], f32, tag="out2")
                    nc.tensor.matmul(psum_o[:], lhsT=s2t[:], rhs=w2_sb[:],
                                     start=True, stop=True)
                    t3 = sb2.tile([P, d_out], f32, tag="ep3")
                    nc.scalar.activation(t3[:], psum_o[:], AF.Copy,
                                         scale=dis_sb[:, g:g + 1])
                    t4 = sb2.tile([P, d_out], f32, tag="ep4")
                    nc.vector.tensor_tensor(t4[:], t3[:], b2_sb[:], op=OP.add)
                    z2t = sb2.tile([P, d_out], f32, tag="z2t")
                    nc.vector.tensor_scalar(out=z2t[:], in0=t4[:],
                                            scalar1=recip_sb[:, g:g + 1], scalar2=None,
                                            op0=OP.mult)
                    nc.sync.dma_start(out=z2_dram[g * P:(g + 1) * P, :], in_=z2t[:])

                # ---------- pool: per group accumulate one-hot matmuls ----------
                for grp in range(pl.n_grp):
                    lo = grp * pl.G_CH
                    hi = min((grp + 1) * pl.G_CH, n_blk)
                    psum_p = ps_p.tile([P, d_out], f32, tag="pool")
                    for j, cblk in enumerate(range(lo, hi)):
                        z2c = sb.tile([P, d_out], f32, tag="z2c")
                        nc.sync.dma_start(out=z2c[:],
                                          in_=z2_dram[cblk * P:(cblk + 1) * P, :])
                        mp = sb.tile([P, P], f32, tag="poolM")
                        nc.vector.tensor_tensor(
                            out=mp[:],
                            in0=pslots_sb[:, cblk:cblk + 1].to_broadcast([P, P]),
                            in1=iof_sb[:], op=OP.is_equal)
                        nc.tensor.matmul(psum_p[:], lhsT=mp[:], rhs=z2c[:],
                                         start=(j == 0), stop=(j == hi - lo - 1))
                    pout = sb.tile([P, d_out], f32, tag="pout")
                    nc.vector.tensor_copy(out=pout[:], in_=psum_p[:])
                    nc.sync.dma_start(out=pool_part[grp * P:(grp + 1) * P, :],
                                      in_=pout[:])

    nc.compile()
    return nc


def make_in_maps(pl):
    maps = []
    for k in range(pl.n_cores):
        maps.append({
            "x_sh": pl.x_sh[k],
            "w1t": pl.w1t,
            "w2": pl.w2_sb,
            "b1b": pl.b1b,
            "b2b": pl.b2b,
            "iotab": pl.iotab,
            "iotaf": pl.iotaf,
            "dis_t": pl.dis_t[k],
            "recip_t": pl.recip_t[k],
            "idx_all": pl.idx_all[k],
            "slots_all": pl.slots_all[k],
            "pool_slots": pl.pool_slots[k],
            "ident": pl.ident,
        })
    return maps


def combine_outputs(pl, parts):
    """parts: list (per core) of pool_part arrays [n_grp*P, d_out]."""
    out = np.zeros((pl.num_graphs, pl.d_out), dtype=np.float32)
    for k in range(pl.n_cores):
        pp = np.asarray(parts[k], dtype=np.float32).reshape(pl.n_grp, P, pl.d_out)
        for g in range(pl.n_grp):
            base = int(pl.pool_base[k, g])
            n = min(P, pl.num_graphs - base)
            if n > 0:
                out[base:base + n] += pp[g, :n]
    return out


def make_pjrt_runner(nc, in_maps, n_cores):
    """Build a jitted 8-core runner (mirrors bass2jax.run_bass_via_pjrt, but
    without donation so the executable can be re-invoked for timing).

    Returns (fn, args, out_names, out_shapes): call fn(*args) -> tuple of
    concatenated per-core outputs.
    """
    import jax
    import numpy as np
    from jax.sharding import Mesh, PartitionSpec
    from jax.experimental.shard_map import shard_map
    import concourse.mybir as mybir
    from concourse.bass2jax import (
        _bass_exec_p, install_neuronx_cc_hook, partition_id_tensor)

    install_neuronx_cc_hook()
    assert nc.dbg_addr is None or not nc.dbg_callbacks

    partition_name = nc.partition_id_tensor.name if nc.partition_id_tensor else None
    in_names, out_names, out_avals, zero_outs = [], [], [], []
    for alloc in nc.m.functions[0].allocations:
        if not isinstance(alloc, mybir.MemoryLocationSet):
            continue
        name = alloc.memorylocations[0].name
        if alloc.kind == "ExternalInput":
            if name != partition_name:
                in_names.append(name)
        elif alloc.kind == "ExternalOutput":
            shape = tuple(alloc.tensor_shape)
            dtype = mybir.dt.np(alloc.dtype)
            out_names.append(name)
            out_avals.append(jax.core.ShapedArray(shape, dtype))
            zero_outs.append(np.zeros(shape, dtype))
    n_params = len(in_names)
    all_names = list(in_names) + list(out_names)
    if partition_name is not None:
        all_names.append(partition_name)

    def _body(*args):
        operands = list(args)
        if partition_name is not None:
            operands.append(partition_id_tensor())
        outs = _bass_exec_p.bind(
            *operands,
            out_avals=tuple(out_avals),
            in_names=tuple(all_names),
            out_names=tuple(out_names),
            lowering_input_output_aliases=(),
            sim_require_finite=True,
            sim_require_nnan=True,
            nc=nc,
        )
        return tuple(outs)

    devices = jax.devices()[:n_cores]
    mesh = Mesh(np.asarray(devices), ("core",))
    n_outs = len(out_names)
    in_specs = (PartitionSpec("core"),) * (n_params + n_outs)
    out_specs = (PartitionSpec("core"),) * n_outs
    fn = jax.jit(shard_map(_body, mesh=mesh, in_specs=in_specs,
                           out_specs=out_specs, check_rep=False),
                 keep_unused=True)
    per_core = [[np.asarray(m[name]) for name in in_names] for m in in_maps]
    concat_in = [np.concatenate([per_core[c][i] for c in range(n_cores)], axis=0)
                 for i in range(n_params)]
    concat_zeros = [np.zeros((n_cores * z.shape[0], *z.shape[1:]), z.dtype)
                    for z in zero_outs]
    args = concat_in + concat_zeros
    out_shapes = [a.shape for a in out_avals]
    return fn, args, out_names, out_shapes


def prepare(x, W1, b1, W2, b2, edge_index, batch):
    pl = make_plan(x, W1, b1, W2, b2, edge_index, batch)
    nc = build_program(pl)
    in_maps = make_in_maps(pl)
    return pl, nc, in_maps


def kernel(x, W1, b1, W2, b2, edge_index, batch):
    from concourse.bass_utils import run_bass_kernel_spmd

    pl, nc, in_maps = prepare(x, W1, b1, W2, b2, edge_index, batch)
    res = run_bass_kernel_spmd(nc, in_maps, list(range(pl.n_cores)))
    parts = [res.results[k]["pool_part"] for k in range(pl.n_cores)]
    return combine_outputs(pl, parts)

